# revision 3
# baseline (speedup 1.0000x reference)
"""BitStackLinear Trainium2 kernel.

y = x @ w.T with w = sum_b sign_b * (u_b @ vt_b), signs bit-packed in qweight.

Strategy: column-parallel over out_features across 8 NeuronCores. Each core:
  Phase A: unpack its sign shard and build w_shard in bf16 on-chip
           (DVE bit-extract + fused (bits-0.5)*2lr + add tree; low-rank
           u@vt via 4 concurrent K=16 row-group matmuls on PE).
  Phase B: y_shard = x @ w_shard.T as bf16 PE matmuls accumulating over
           the 4096-long contraction in PSUM.

The contraction index is permuted bit-plane-major (i' = (i%8)*(I/8) + i//8)
so each 128-partition i'-chunk uses a single constant bit position; x and vt
are permuted on the host to match (pure layout change, contraction order is
arbitrary).
"""
import sys

for _p in ("/opt/trn_rl_repo", "/root/.axon_site/_ro/trn_rl_repo"):
    if _p not in sys.path:
        sys.path.insert(0, _p)

import numpy as np
import ml_dtypes

import concourse.bass as bass
import concourse.tile as tile
from concourse import mybir
from concourse.bass_utils import run_bass_kernel_spmd

N_CORES = 8
B = 4       # bit planes
K = 16      # low-rank
T = 2048    # tokens
I = 4096    # in_features
O = 11008   # out_features
O_S = O // N_CORES  # 1376 per core

_OG_MAX = 512  # psum bank free width (f32)


def _og_chunks(o_s):
    out, o0 = [], 0
    while o0 < o_s:
        w = min(_OG_MAX, o_s - o0)
        out.append((o0, w))
        o0 += w
    return out


def build_nc(t=T, i=I, o_s=O_S):
    """Build the per-core SPMD Bass program (identical on all cores)."""
    nb = i // 8          # packed bytes per (b, o)
    mb_n = nb // 128     # byte-row blocks per bit plane
    nc_i = i // 128      # i'-chunks
    assert nc_i == 8 * mb_n and t % 128 == 0
    n_tc = t // 128
    ogs = _og_chunks(o_s)

    nc = bass.Bass("TRN2", target_bir_lowering=False, debug=False)
    xt_d = nc.dram_tensor("xt", [i, t], mybir.dt.bfloat16, kind="ExternalInput")
    qt_d = nc.dram_tensor("qt", [B, nb, o_s], mybir.dt.uint8, kind="ExternalInput")
    vt_d = nc.dram_tensor("vtp", [128, i], mybir.dt.float32, kind="ExternalInput")
    ut_d = nc.dram_tensor("utp", [128, o_s], mybir.dt.float32, kind="ExternalInput")
    y_d = nc.dram_tensor("y", [t, o_s], mybir.dt.float32, kind="ExternalOutput")

    f32 = mybir.dt.float32
    bf16 = mybir.dt.bfloat16
    SHR = mybir.AluOpType.logical_shift_right
    AND = mybir.AluOpType.bitwise_and
    ADD = mybir.AluOpType.add
    MUL = mybir.AluOpType.mult

    with tile.TileContext(nc) as tc:
        with (
            tc.tile_pool(name="const", bufs=1) as cpool,
            tc.tile_pool(name="w", bufs=1) as wpool,
            tc.tile_pool(name="bits", bufs=2) as bitspool,
            tc.tile_pool(name="lrsb", bufs=2) as lrsbpool,
            tc.tile_pool(name="m", bufs=2) as mpool,
            tc.tile_pool(name="tmp", bufs=2) as tmppool,
            tc.tile_pool(name="x", bufs=4) as xpool,
            tc.tile_pool(name="ysb", bufs=2) as ysbpool,
            tc.tile_pool(name="lrps", bufs=1, space="PSUM") as lrps,
            tc.tile_pool(name="yps", bufs=1, space="PSUM") as yps,
        ):
            # ---- persistent loads ----
            q_sb = cpool.tile([128, B * mb_n * o_s], mybir.dt.uint8, tag="q")
            for b in range(B):
                for mb in range(mb_n):
                    s = (b * mb_n + mb) * o_s
                    nc.sync.dma_start(
                        q_sb[:, s:s + o_s], qt_d.ap()[b, mb * 128:(mb + 1) * 128, :]
                    )
            vt_sb = cpool.tile([128, i], f32, tag="vt")
            nc.sync.dma_start(vt_sb[:], vt_d.ap())
            ut_sb = cpool.tile([128, o_s], f32, tag="ut")
            nc.sync.dma_start(ut_sb[:], ut_d.ap())
            w_sb = wpool.tile([128, nc_i * o_s], bf16, tag="w")

            # ---- Phase A: build w (bf16, [i' chunks of 128, o_s]) ----
            for c in range(nc_i):
                j, mb = c // mb_n, c % mb_n
                bits = bitspool.tile([128, B * o_s], mybir.dt.uint8, tag="bits")
                for b in range(B):
                    qs = (b * mb_n + mb) * o_s
                    nc.vector.tensor_scalar(
                        bits[:, b * o_s:(b + 1) * o_s],
                        q_sb[:, qs:qs + o_s], j, 1, SHR, AND,
                    )
                for (o0, ow) in ogs:
                    lr_t = [
                        lrps.tile([128, _OG_MAX], f32, name=f"lr{b}", tag=f"lr{b}") for b in range(B)
                    ]
                    for b in range(B):
                        nc.tensor.matmul(
                            lr_t[b][:, :ow],
                            vt_sb[32 * b:32 * b + K, c * 128:(c + 1) * 128],
                            ut_sb[32 * b:32 * b + K, o0:o0 + ow],
                            start=True, stop=True, tile_position=(32 * b, 0),
                        )
                    lr_sb = lrsbpool.tile([128, B * _OG_MAX], bf16, tag="lrsb")
                    for b in range(B):
                        nc.scalar.copy(
                            lr_sb[:, b * _OG_MAX:b * _OG_MAX + ow], lr_t[b][:, :ow]
                        )
                    m_t = mpool.tile([128, B * _OG_MAX], bf16, tag="m")
                    for b in range(B):
                        nc.vector.scalar_tensor_tensor(
                            m_t[:, b * _OG_MAX:b * _OG_MAX + ow],
                            bits[:, b * o_s + o0:b * o_s + o0 + ow],
                            -0.5,
                            lr_sb[:, b * _OG_MAX:b * _OG_MAX + ow],
                            ADD, MUL,
                        )
                    t01 = tmppool.tile([128, _OG_MAX], bf16, tag="t01")
                    t23 = tmppool.tile([128, _OG_MAX], bf16, tag="t23")
                    nc.vector.tensor_add(
                        t01[:, :ow], m_t[:, 0:ow],
                        m_t[:, _OG_MAX:_OG_MAX + ow],
                    )
                    nc.vector.tensor_add(
                        t23[:, :ow], m_t[:, 2 * _OG_MAX:2 * _OG_MAX + ow],
                        m_t[:, 3 * _OG_MAX:3 * _OG_MAX + ow],
                    )
                    nc.vector.tensor_add(
                        w_sb[:, c * o_s + o0:c * o_s + o0 + ow],
                        t01[:, :ow], t23[:, :ow],
                    )

            # ---- Phase B: y = xt.T @ w, accumulate over i'-chunks ----
            for (o0, ow) in ogs:
                tc0 = 0
                while tc0 < n_tc:
                    g = min(4, n_tc - tc0)
                    y_t = [
                        yps.tile([128, _OG_MAX], f32, name=f"yt{u}", tag=f"y{u}") for u in range(g)
                    ]
                    for c in range(nc_i):
                        xt_t = xpool.tile([128, 128 * g], bf16, tag="x")
                        nc.sync.dma_start(
                            xt_t[:],
                            xt_d.ap()[c * 128:(c + 1) * 128,
                                      tc0 * 128:(tc0 + g) * 128],
                        )
                        for u in range(g):
                            nc.tensor.matmul(
                                y_t[u][:, :ow],
                                xt_t[:, u * 128:(u + 1) * 128],
                                w_sb[:, c * o_s + o0:c * o_s + o0 + ow],
                                start=(c == 0), stop=(c == nc_i - 1),
                            )
                    for u in range(g):
                        y_sb = ysbpool.tile([128, _OG_MAX], f32, tag="ysb")
                        nc.scalar.copy(y_sb[:, :ow], y_t[u][:, :ow])
                        nc.sync.dma_start(
                            y_d.ap()[(tc0 + u) * 128:(tc0 + u + 1) * 128,
                                     o0:o0 + ow],
                            y_sb[:, :ow],
                        )
                    tc0 += g

    _split_waits(nc)
    return nc


def _split_waits(nc, maxw=1):
    """This walrus build rejects instructions with more than a couple of
    sync-wait commands; move excess waits onto preceding same-engine NoOps."""
    for bb in nc.m.functions[0].blocks:
        insts = bb.instructions
        idx = 0
        while idx < len(insts):
            ins = insts[idx]
            si = ins.sync_info
            if si is not None and len(si.on_wait) > maxw:
                waits = list(si.on_wait)
                extra, keep = waits[:-maxw], waits[-maxw:]
                nops = []
                for k, wt in enumerate(extra):
                    nops.append(mybir.InstNoOp(
                        name=f"{ins.name}-wsplit{k}",
                        engine=ins.engine,
                        bass_nofuse=True,
                        sync_info=mybir.SyncInfo(on_wait=[wt], on_update=[]),
                    ))
                ins.sync_info = mybir.SyncInfo(on_wait=keep,
                                               on_update=list(si.on_update))
                for k, nop in enumerate(nops):
                    nc.register_instruction(nop, overwrite=True)
                    insts.insert(idx + k, nop)
                idx += len(nops)
            idx += 1


def prep_inputs(x, qweight, u, vt, n_cores=N_CORES):
    """Host-side layout prep + sharding. Returns (in_maps, meta)."""
    t, i = x.shape
    b_, o, k_ = u.shape
    nb = i // 8
    o_s = o // n_cores

    # x -> xt[i', t] bf16 with i' = j*(i/8) + m  (j-major bit-plane order)
    xt = np.ascontiguousarray(
        x.T.reshape(nb, 8, t).transpose(1, 0, 2).reshape(i, t)
    ).astype(ml_dtypes.bfloat16)

    # qweight -> qt[b, m, o] uint8 (byte-transposed)
    qt = np.ascontiguousarray(
        qweight.astype(np.uint8).reshape(b_, o, nb).transpose(0, 2, 1)
    )

    # vt -> permuted + stacked into PE row groups [128, i]
    vtp = vt.reshape(b_, k_, nb, 8).transpose(0, 1, 3, 2).reshape(b_, k_, i)
    vt_stack = np.zeros((128, i), np.float32)
    for b in range(b_):
        vt_stack[32 * b:32 * b + k_, :] = vtp[b]

    # u -> 2*u^T stacked [128, o] (x2 folds the (bits-0.5)*2lr identity)
    ut_full = np.zeros((128, o), np.float32)
    for b in range(b_):
        ut_full[32 * b:32 * b + k_, :] = 2.0 * u[b].T

    in_maps = []
    for core in range(n_cores):
        o0 = core * o_s
        in_maps.append({
            "xt": xt,
            "qt": np.ascontiguousarray(qt[:, :, o0:o0 + o_s]),
            "vtp": vt_stack,
            "utp": np.ascontiguousarray(ut_full[:, o0:o0 + o_s]),
        })
    return in_maps, (t, i, o, o_s)


_NC_CACHE = {}


def _get_nc(t, i, o_s):
    key = (t, i, o_s)
    if key not in _NC_CACHE:
        _NC_CACHE[key] = build_nc(t, i, o_s)
    return _NC_CACHE[key]


def run(x, qweight, u, vt, trace=False, **spmd_kwargs):
    in_maps, (t, i, o, o_s) = prep_inputs(x, qweight, u, vt)
    nc = _get_nc(t, i, o_s)
    res = run_bass_kernel_spmd(
        nc, in_maps, list(range(N_CORES)), trace=trace, **spmd_kwargs
    )
    y = np.concatenate([res.results[c]["y"] for c in range(N_CORES)], axis=1)
    return y, res


def kernel(x, qweight, u, vt):
    x = np.asarray(x, dtype=np.float32)
    qweight = np.asarray(qweight)
    u = np.asarray(u, dtype=np.float32)
    vt = np.asarray(vt, dtype=np.float32)
    y, _ = run(x, qweight, u, vt, trace=False)
    return y


# revision 22
# speedup vs baseline: 1.6395x; 1.6395x over previous
"""BitStackLinear Trainium2 kernel.

y = x @ w.T with w = sum_b sign_b * (u_b @ vt_b), signs bit-packed in qweight.

Strategy: column-parallel over out_features across 8 NeuronCores. Each core
pipelines two phases per 512-wide out-feature group (og):
  Phase A (build w[:, og] bf16, one 128-row i'-chunk at a time):
    - low-rank lr_b = u_b @ vt_b via 4 concurrent K=16 row-group matmuls (PE)
    - ACT copies lr from PSUM to SBUF bf16
    - DVE isolates bit j ((q & (1<<j)) -> {0, 2^j}), ACT Sign(x - 2^(j-1))
      gives +-1 bf16, DVE/GPSIMD multiply + add tree -> w chunk
  Phase B (y[:, og] = x @ w[:, og].T): four token-chunk accumulators run
    concurrently in four PSUM banks, consuming w chunks in the same order
    phase A produces them - so B trickles right behind A and the PE stream
    stays dense across og groups.

The contraction index is permuted bit-plane-major (i' = (i%8)*(I/8) + i//8)
so each 128-partition i'-chunk uses a single constant bit position; x and vt
are permuted on the host to match (pure layout change, contraction order is
arbitrary).
"""
import sys

for _p in ("/opt/trn_rl_repo", "/root/.axon_site/_ro/trn_rl_repo"):
    if _p not in sys.path:
        sys.path.insert(0, _p)

import numpy as np
import ml_dtypes

import concourse.bass as bass
import concourse.tile as tile
from concourse import mybir
from concourse.bass_utils import run_bass_kernel_spmd

N_CORES = 8
B = 4       # bit planes
K = 16      # low-rank
T = 2048    # tokens
I = 4096    # in_features
O = 11008   # out_features
O_S = O // N_CORES  # 1376 per core

_SLOT = 512  # psum bank free width (f32)


def _og_chunks(o_s):
    out, o0 = [], 0
    while o0 < o_s:
        w = min(_SLOT, o_s - o0)
        out.append((o0, w))
        o0 += w
    return out


def build_nc(t=T, i=I, o_s=O_S):
    """Build the per-core SPMD Bass program (identical on all cores)."""
    nb = i // 8          # packed words per (b, o)
    mb_n = nb // 128     # byte-row blocks per bit plane
    nc_i = i // 128      # i'-chunks
    assert nc_i == 8 * mb_n and t % 512 == 0
    n_tc = t // 128
    n_tq = n_tc // 4     # token quads (4 chunks of 128 -> one 512-wide xt tile)
    ogs = _og_chunks(o_s)

    # phase A production order = phase B consumption order
    c_order = [j * mb_n + mb for mb in range(mb_n) for j in range(8)]

    nc = bass.Bass("TRN2", target_bir_lowering=False, debug=False)
    # Sign bias constants (-2^(j-1)); activation() resolves float biases
    # through the const-AP database
    for jj in range(8):
        vv = -float(2 ** jj) / 2.0
        th = nc.alloc_sbuf_tensor(f"const-bias-{jj}", [128, 1], mybir.dt.float32)
        nc.gpsimd.memset(th.ap(), vv)
        nc.const_aps.aps[(mybir.dt.float32, vv)] = th.ap()
    nc.all_engine_barrier()

    xt_d = nc.dram_tensor("xt", [i, t], mybir.dt.bfloat16, kind="ExternalInput")
    qt_d = nc.dram_tensor("qt", [B, nb, o_s], mybir.dt.uint16, kind="ExternalInput")
    vt_d = nc.dram_tensor("vtp", [128, i], mybir.dt.bfloat16, kind="ExternalInput")
    ut_d = nc.dram_tensor("utp", [128, o_s], mybir.dt.bfloat16, kind="ExternalInput")
    y_d = nc.dram_tensor("y", [t, o_s], mybir.dt.float32, kind="ExternalOutput")

    f32 = mybir.dt.float32
    bf16 = mybir.dt.bfloat16
    u16 = mybir.dt.uint16
    AND = mybir.AluOpType.bitwise_and
    SIGN = mybir.ActivationFunctionType.Sign

    with tile.TileContext(nc) as tc:
        with (
            tc.tile_pool(name="const", bufs=1) as cpool,
            tc.tile_pool(name="w", bufs=1) as wpool,
            tc.tile_pool(name="q", bufs=2) as qpool,
            tc.tile_pool(name="andt", bufs=3) as andpool,
            tc.tile_pool(name="sgn", bufs=4) as sgnpool,
            tc.tile_pool(name="lrsb", bufs=4) as lrsbpool,
            tc.tile_pool(name="m", bufs=4) as mpool,
            tc.tile_pool(name="tmp", bufs=3) as tmppool,
            tc.tile_pool(name="x", bufs=10) as xpool,
            tc.tile_pool(name="ysb", bufs=3) as ysbpool,
            tc.tile_pool(name="lrps", bufs=1, space="PSUM") as lrps,
            tc.tile_pool(name="yps", bufs=1, space="PSUM") as yps,
        ):
            # ---- persistent loads ----
            vt_sb = cpool.tile([128, i], bf16, tag="vt")
            nc.sync.dma_start(vt_sb[:], vt_d.ap())
            ut_sb = cpool.tile([128, o_s], bf16, tag="ut")
            nc.sync.dma_start(ut_sb[:], ut_d.ap())
            q_sb = []
            for mb in range(mb_n):
                q_t = qpool.tile([128, B * o_s], u16, name=f"q{mb}",
                                 tag=f"q{mb % 2}")
                for b in range(B):
                    nc.sync.dma_start(
                        q_t[:, b * o_s:(b + 1) * o_s],
                        qt_d.ap()[b, mb * 128:(mb + 1) * 128, :],
                    )
                q_sb.append(q_t)

            for ogi, (o0, ow) in enumerate(ogs):
                # w[:, og] block, [128, nc_i * ow] (chunk-major)
                w_og = wpool.tile([128, nc_i * _SLOT], bf16, name=f"w{ogi}",
                                  tag=f"w{ogi % 2}")

                # ---- Phase A for this og ----
                for ci, c in enumerate(c_order):
                    j, mb = c // mb_n, c % mb_n
                    q_t = q_sb[mb]
                    # bit j isolated: {0, 2^j} u16 (bitwise AND), then a
                    # {0,1} bf16 indicator via is_gt; -0.5 is fused into the
                    # multiply and the factor 2 into utp
                    and_t = andpool.tile([128, B * _SLOT], u16, tag="andt")
                    nc.vector.tensor_scalar(
                        and_t[:, : B * ow].rearrange(
                            "p (b w) -> p b w", b=B),
                        q_t[:].rearrange(
                            "p (b w) -> p b w", b=B)[:, :, o0:o0 + ow],
                        1 << j, None, AND,
                    )
                    s_t = sgnpool.tile([128, B * _SLOT], bf16, tag="sgn")
                    nc.vector.tensor_scalar(
                        s_t[:, : B * ow], and_t[:, : B * ow], 0.0, 0.5,
                        mybir.AluOpType.is_gt, mybir.AluOpType.subtract,
                    )
                    # low-rank: 4 concurrent K=16 row-group matmuls into
                    # one 4-bank psum tile, evicted with one wide ACT copy
                    lr_ps = lrps.tile([128, B * _SLOT], f32, name="lr_ps",
                                      tag="lr_ps")
                    for b in range(B):
                        nc.tensor.matmul(
                            lr_ps[:, b * _SLOT:b * _SLOT + ow],
                            vt_sb[32 * b:32 * b + K, c * 128:(c + 1) * 128],
                            ut_sb[32 * b:32 * b + K, o0:o0 + ow],
                            start=True, stop=True, tile_position=(32 * b, 0),
                        )
                    lr_sb = lrsbpool.tile([128, B * _SLOT], bf16, tag="lrsb")
                    if ow == _SLOT:
                        nc.scalar.copy(lr_sb[:], lr_ps[:])
                    else:
                        nc.scalar.copy(
                            lr_sb[:].rearrange(
                                "p (b w) -> p b w", b=B)[:, :, :ow],
                            lr_ps[:].rearrange(
                                "p (b w) -> p b w", b=B)[:, :, :ow],
                        )
                    # m_b = s_b * lr_b; plane 3 multiply + pair-sum on
                    # gpsimd to offload DVE
                    m_t = mpool.tile([128, B * _SLOT], bf16, tag="m")
                    for b in range(4):
                        nc.vector.tensor_mul(
                            m_t[:, b * _SLOT:b * _SLOT + ow],
                            s_t[:, b * ow:(b + 1) * ow],
                            lr_sb[:, b * _SLOT:b * _SLOT + ow],
                        )
                    t01 = tmppool.tile([128, _SLOT], bf16, tag="t01")
                    t23 = tmppool.tile([128, _SLOT], bf16, tag="t23")
                    nc.vector.tensor_add(
                        t01[:, :ow], m_t[:, 0:ow], m_t[:, _SLOT:_SLOT + ow],
                    )
                    nc.vector.tensor_add(
                        t23[:, :ow], m_t[:, 2 * _SLOT:2 * _SLOT + ow],
                        m_t[:, 3 * _SLOT:3 * _SLOT + ow],
                    )
                    nc.vector.tensor_add(
                        w_og[:, ci * _SLOT:ci * _SLOT + ow],
                        t01[:, :ow], t23[:, :ow],
                    )

                # ---- Phase B for this og: 4 concurrent token quads ----
                for tq in range(n_tq):
                    ysums = [
                        yps.tile([128, _SLOT], f32, name=f"ysum{u}",
                                 tag=f"y{u}")
                        for u in range(4)
                    ]
                    for ci, c in enumerate(c_order):
                        xt_t = xpool.tile([128, 512], bf16, tag="x")
                        nc.sync.dma_start(
                            xt_t[:],
                            xt_d.ap()[c * 128:(c + 1) * 128,
                                      tq * 512:(tq + 1) * 512],
                        )
                        for u in range(4):
                            nc.tensor.matmul(
                                ysums[u][:, :ow],
                                xt_t[:, u * 128:(u + 1) * 128],
                                w_og[:, ci * _SLOT:ci * _SLOT + ow],
                                start=(ci == 0), stop=(ci == nc_i - 1),
                            )
                    for u in range(4):
                        y_sb = ysbpool.tile([128, _SLOT], f32, tag="ysb")
                        nc.scalar.copy(y_sb[:, :ow], ysums[u][:, :ow])
                        nc.sync.dma_start(
                            y_d.ap()[(tq * 4 + u) * 128:(tq * 4 + u + 1) * 128,
                                     o0:o0 + ow],
                            y_sb[:, :ow],
                        )

    _split_waits(nc)
    return nc


def _split_waits(nc, maxw=1):
    """This walrus build rejects instructions with more than a couple of
    sync-wait commands; move excess waits onto preceding same-engine NoOps."""
    for bb in nc.m.functions[0].blocks:
        insts = bb.instructions
        idx = 0
        while idx < len(insts):
            ins = insts[idx]
            si = ins.sync_info
            if si is not None and len(si.on_wait) > maxw:
                waits = list(si.on_wait)
                extra, keep = waits[:-maxw], waits[-maxw:]
                nops = []
                for k, wt in enumerate(extra):
                    nops.append(mybir.InstNoOp(
                        name=f"{ins.name}-wsplit{k}",
                        engine=ins.engine,
                        bass_nofuse=True,
                        sync_info=mybir.SyncInfo(on_wait=[wt], on_update=[]),
                    ))
                ins.sync_info = mybir.SyncInfo(on_wait=keep,
                                               on_update=list(si.on_update))
                for k, nop in enumerate(nops):
                    nc.register_instruction(nop, overwrite=True)
                    insts.insert(idx + k, nop)
                idx += len(nops)
            idx += 1


def prep_inputs(x, qweight, u, vt, n_cores=N_CORES):
    """Host-side layout prep + sharding. Returns (in_maps, meta)."""
    t, i = x.shape
    b_, o, k_ = u.shape
    nb = i // 8
    o_s = o // n_cores

    # x -> xt[i', t] bf16 with i' = j*(i/8) + m  (j-major bit-plane order)
    xt = np.ascontiguousarray(
        x.T.reshape(nb, 8, t).transpose(1, 0, 2).reshape(i, t)
    ).astype(ml_dtypes.bfloat16)

    # qweight -> qt[b, m, o] uint16 (byte-transposed; u16 ops hit the DVE
    # 16-bit packed mode, 2x the u8 rate)
    qt = np.ascontiguousarray(
        qweight.astype(np.uint16).reshape(b_, o, nb).transpose(0, 2, 1)
    )

    # vt -> permuted + stacked into PE row groups [128, i], bf16
    vtp = vt.reshape(b_, k_, nb, 8).transpose(0, 1, 3, 2).reshape(b_, k_, i)
    vt_stack = np.zeros((128, i), np.float32)
    for b in range(b_):
        vt_stack[32 * b:32 * b + k_, :] = vtp[b]
    vt_stack = vt_stack.astype(ml_dtypes.bfloat16)

    # u -> u^T stacked [128, o], bf16
    ut_full = np.zeros((128, o), np.float32)
    for b in range(b_):
        ut_full[32 * b:32 * b + k_, :] = 2.0 * u[b].T
    ut_full = ut_full.astype(ml_dtypes.bfloat16)

    in_maps = []
    for core in range(n_cores):
        o0 = core * o_s
        in_maps.append({
            "xt": xt,
            "qt": np.ascontiguousarray(qt[:, :, o0:o0 + o_s]),
            "vtp": vt_stack,
            "utp": np.ascontiguousarray(ut_full[:, o0:o0 + o_s]),
        })
    return in_maps, (t, i, o, o_s)


_NC_CACHE = {}


def _get_nc(t, i, o_s):
    key = (t, i, o_s)
    if key not in _NC_CACHE:
        _NC_CACHE[key] = build_nc(t, i, o_s)
    return _NC_CACHE[key]


def run(x, qweight, u, vt, trace=False, **spmd_kwargs):
    in_maps, (t, i, o, o_s) = prep_inputs(x, qweight, u, vt)
    nc = _get_nc(t, i, o_s)
    res = run_bass_kernel_spmd(
        nc, in_maps, list(range(N_CORES)), trace=trace, **spmd_kwargs
    )
    y = np.concatenate([res.results[c]["y"] for c in range(N_CORES)], axis=1)
    return y, res


def kernel(x, qweight, u, vt):
    x = np.asarray(x, dtype=np.float32)
    qweight = np.asarray(qweight)
    u = np.asarray(u, dtype=np.float32)
    vt = np.asarray(vt, dtype=np.float32)
    y, _ = run(x, qweight, u, vt, trace=False)
    return y
